# revision 31
# baseline (speedup 1.0000x reference)
"""GPT-OSS expert MLP (gate/up GEMM + clamped GLU + down GEMM + routing
scale) on 8 Trainium2 NeuronCores.

Sharding: tensor-parallel split of the intermediate dim I=2880 across 8
cores (360 columns each, padded to 384 = 3*128). Each core computes
  gate/up = hidden @ W[:, slice] ; glu ; y_partial = glu_h @ down_w[slice, :]
and writes its full [H, T_active] partial (transposed layout). The host
sums the 8 partials, applies down bias, routing weights, and the residual.

Key optimizations (all HW-microbenchmarked):
- Token compaction: tokens whose routing-weight sum is zero contribute
  exactly final_hidden_states, so only the ~445 active tokens are shipped
  and computed (moving dim 448 instead of 512).
- FP8 weights: the quantized weights (values k/32, |k| <= 4) are exactly
  representable in float8e4, and the PE accepts mixed fp8 lhsT x bf16 rhs
  at full rate -> half the weight DMA and less datapath toggle power
  (the PE is power-throttled on real data: identical programs run 63%
  slower on random data than on all-ones).
- Double-buffered streaming (bufs=2) so body i+1's loads prefetch during
  body i with no WAR stall; one big DMA per tensor; stores on the ACT
  HWDGE ring so the SP ring only carries loads.
- The timing loop interleaves the down GEMM of body i between the six
  long gate/up accumulation chains of body i+1: short 3-matmul PSUM
  chains each followed by a PSUM->SBUF copy throttle the PE when run
  back-to-back, but cost ~nothing when spaced between 23-matmul chains.
- hidden_states stay bf16 (fp8 activations would give ~3% error vs the
  2% gate); PSUM accumulates fp32; partials sum on the host in fp64.
"""

import numpy as np
import ml_dtypes

BF16 = ml_dtypes.bfloat16
FP8 = ml_dtypes.float8_e4m3fn

H = 2880          # hidden size
I = 2880          # intermediate size
T = 512           # tokens (full problem size)
NCORES = 8
IC = I // NCORES  # 360 intermediate cols per core
ICP = 384         # padded to 3 * 128
MT = ICP // 128   # 3 i-tiles per core
HP = 2944         # H padded to 23 * 128
KT = HP // 128    # 23 k-tiles over hidden dim
ALPHA = 1.702
LIMIT = 7.0
_cache = {}


def routing_compaction(expert_mask, routing_weights):
    """Tokens with sum_j mask[j,t]*rw[t,j] == 0 contribute exactly
    final_hidden_states[t] to the output, so the device only computes the
    active tokens. Returns (active_idx, tok_w, tp) with tp = active count
    padded up to a multiple of 64 (the compiled moving-dim size)."""
    mask = np.asarray(expert_mask, np.float32)
    rw = np.asarray(routing_weights, np.float32)
    tok_w = np.einsum("jt,tj->t", mask, rw)
    active = np.flatnonzero(tok_w)
    tp = max(64, -(-len(active) // 64) * 64)
    return active, tok_w, tp


def build_program(loop_reps=None, unroll_bodies=None, unroll=16, tp=448,
                  probe=None, big_loads=True, mix=True, py_bufs=4):
    """Build (and compile) the per-core Bass program. Identical on all cores;
    per-core data comes from in_maps.

    Structure (from HW microbenchmarks):
    - All streamed inputs are double-buffered (bufs=2), so body i+1's loads
      prefetch during body i with no WAR stall.
    - The down GEMM of body i is emitted interleaved between the six long
      gate/up accumulation chains of body i+1: short 3-matmul down chains
      with a PSUM->SBUF copy per chain throttle the PE to ~253 ns per
      448-col matmul when run back-to-back, but cost ~197 ns when spaced
      between 23-matmul chains (copy/drain interference amortizes).
    - Stores ride the ACT HWDGE ring so the SP ring only carries loads.
    loop_reps wraps `unroll` bodies in a For_i (timing only);
    unroll_bodies=N emits N bodies with no loop (sim only).
    """
    import concourse.bacc as bacc
    import concourse.mybir as mybir
    import concourse.tile as tile

    fp32 = mybir.dt.float32
    bf16 = mybir.dt.bfloat16
    fp8 = mybir.dt.float8e4

    nc = bacc.Bacc("TRN2", target_bir_lowering=False, debug=False,
                   num_devices=NCORES)

    TT = tp            # active-token count (moving dim)
    hid_d = nc.dram_tensor("hid", [128, KT * TT], bf16, kind="ExternalInput").ap()
    gu_d = nc.dram_tensor("gu", [128, 2 * MT * KT * 128], fp8,
                          kind="ExternalInput").ap()
    dw_d = nc.dram_tensor("dw", [128, KT * MT * 128], fp8,
                          kind="ExternalInput").ap()
    gb_d = nc.dram_tensor("gb", [128, MT], fp32, kind="ExternalInput").ap()
    ub_d = nc.dram_tensor("ub", [128, MT], fp32, kind="ExternalInput").ap()
    y_d = nc.dram_tensor("y", [HP, TT], bf16, kind="ExternalOutput").ap()

    # down h-tile groups: one store per group, interleaved 1:1 with the six
    # gate/up chains of the next body
    DGROUPS = [4, 4, 4, 4, 4, 3]
    do_loads = probe in (None, 'loads', 'nostores')
    do_compute = probe in (None, 'nostores', 'pe', 'pe0')
    do_stores = probe in (None, 'pe', 'pe0')
    pe_mode = probe in ('pe', 'pe0')
    pin_ops = probe == 'pe0'

    def emit_loads(pools, do_dma=None):
        """One big DMA per tensor: with bufs=2 the whole next body is
        prefetched during the current one, so arrival granularity no longer
        matters and fewer DMAs minimize HWDGE issue/completion overhead."""
        do_loads_ = do_loads if do_dma is None else do_dma
        wpool = pools[0]
        hid_t = wpool.tile([128, KT * TT], bf16, tag="hid", name="hid")
        gu_t = {}
        gb_t = wpool.tile([128, MT], fp32, tag="gb", name="gb")
        ub_t = wpool.tile([128, MT], fp32, tag="ub", name="ub")
        for grp in range(6):
            gu_t[grp] = wpool.tile([128, KT * 128], fp8, tag=f"gu{grp}",
                                   name=f"gu{grp}")
        dw_t = wpool.tile([128, KT * MT * 128], fp8, tag="dw", name="dw")
        if do_loads_ and big_loads:
            nc.sync.dma_start(gu_t[0][:], gu_d[:, 0:KT * 128])
            nc.sync.dma_start(hid_t[:], hid_d[:])
            nc.sync.dma_start(gu_t[1][:], gu_d[:, KT * 128:2 * KT * 128])
            nc.sync.dma_start(gb_t[:], gb_d[:])
            nc.sync.dma_start(ub_t[:], ub_d[:])
            for grp in range(2, 6):
                nc.sync.dma_start(gu_t[grp][:],
                                  gu_d[:, grp * KT * 128:(grp + 1) * KT * 128])
            nc.sync.dma_start(dw_t[:], dw_d[:])
        elif do_loads_:
            # pieces: interleave hid/gu chunks like the PE consumes them
            hp = [0, 3, 6, 9, 12, 15, 18, 21, 23]
            gp = [0, 6, 12, 18, 23]
            for j in range(4):
                nc.sync.dma_start(
                    gu_t[0][:, gp[j] * 128:gp[j + 1] * 128],
                    gu_d[:, gp[j] * 128:gp[j + 1] * 128])
                nc.sync.dma_start(hid_t[:, hp[j] * TT:hp[j + 1] * TT],
                                  hid_d[:, hp[j] * TT:hp[j + 1] * TT])
            nc.sync.dma_start(gb_t[:], gb_d[:])
            nc.sync.dma_start(ub_t[:], ub_d[:])
            for j in range(4):
                nc.sync.dma_start(
                    gu_t[1][:, gp[j] * 128:gp[j + 1] * 128],
                    gu_d[:, KT * 128 + gp[j] * 128:KT * 128 + gp[j + 1] * 128])
                nc.sync.dma_start(hid_t[:, hp[j + 4] * TT:hp[j + 5] * TT],
                                  hid_d[:, hp[j + 4] * TT:hp[j + 5] * TT])
            for grp in range(2, 6):
                for j in (0, 1):
                    half = [0, 12, 23]
                    nc.sync.dma_start(
                        gu_t[grp][:, half[j] * 128:half[j + 1] * 128],
                        gu_d[:, grp * KT * 128 + half[j] * 128:
                             grp * KT * 128 + half[j + 1] * 128])
            nc.sync.dma_start(dw_t[:], dw_d[:])

        hglu = pools[2].tile([128, MT * TT], bf16, tag="hglu", name="hglu")
        return {"hid": hid_t, "gu": gu_t, "gb": gb_t, "ub": ub_t,
                "dw": dw_t, "hglu": hglu, "nch": 0, "yo": None}

    def emit_gu_chain(pools, h, c):
        """Chain c of 6: even = gate chain of m=c//2, odd = up chain + GLU."""
        wpool, glupool, hglupool, psum, psum_y, ypool = pools
        m = c // 2

        def rhs(kt):
            if pin_ops:
                kt = 0
            return h["hid"][:, kt * TT:(kt + 1) * TT]

        def lhsT(grp, kt):
            if pin_ops:
                grp, kt = 0, 0
            return h["gu"][grp][:, kt * 128:(kt + 1) * 128]

        if c % 2 == 0:
            pg = psum.tile([128, TT], fp32, tag="pg", name="pg")
            for kt in range(KT):
                nc.tensor.matmul(pg[:], lhsT(2 * m, kt), rhs(kt),
                                 start=(kt == 0), stop=(kt == KT - 1))
            h["pg"] = pg
        else:
            pu = psum.tile([128, TT], fp32, tag="pu", name="pu")
            for kt in range(KT):
                nc.tensor.matmul(pu[:], lhsT(2 * m + 1, kt), rhs(kt),
                                 start=(kt == 0), stop=(kt == KT - 1))
            pg = h["pg"]
            # gate: g = min(pg + gb, L); sg = silu(ALPHA*g) = ALPHA*glu
            tg = glupool.tile([128, TT], fp32, tag="tg", name="tg")
            nc.vector.tensor_scalar(tg[:], pg[:], h["gb"][:, m:m + 1], LIMIT,
                                    mybir.AluOpType.add, mybir.AluOpType.min)
            sg = glupool.tile([128, TT], fp32, tag="sg", name="sg")
            nc.scalar.activation(sg[:], tg[:],
                                 mybir.ActivationFunctionType.Silu,
                                 scale=ALPHA)
            # up: u = clip(pu + ub, -L, L); tu5 = u + 1
            tu = glupool.tile([128, TT], fp32, tag="tu", name="tu")
            nc.vector.tensor_scalar(tu[:], pu[:], h["ub"][:, m:m + 1], LIMIT,
                                    mybir.AluOpType.add, mybir.AluOpType.min)
            tu3 = glupool.tile([128, TT], fp32, tag="tu3", name="tu3")
            nc.vector.tensor_scalar(tu3[:], tu[:], -LIMIT, 1.0 / ALPHA,
                                    mybir.AluOpType.max, mybir.AluOpType.mult)
            tu4 = glupool.tile([128, TT], fp32, tag="tu4", name="tu4")
            nc.vector.tensor_scalar_add(tu4[:], tu3[:], 1.0 / ALPHA)
            # hglu = (ALPHA*glu) * (u+1)/ALPHA = glu * (u+1); dw stays exact
            nc.vector.tensor_tensor(h["hglu"][:, m * TT:(m + 1) * TT],
                                    sg[:], tu4[:], mybir.AluOpType.mult)

    def emit_down_group(pools, h, g):
        """Down chains for DGROUPS[g] h-tiles + copies + one batched store."""
        wpool, glupool, hglupool, psum, psum_y, ypool = pools
        ht0 = sum(DGROUPS[:g])
        nb = DGROUPS[g]
        yo = ypool.tile([128, nb * TT], bf16, tag=f"yo{nb}", name=f"yo{nb}")
        for bi in range(nb):
            ht = ht0 + bi
            py = psum_y.tile([128, TT], fp32, tag="py", name="py")
            for it in range(MT):
                if pin_ops:
                    dsl = h["dw"][:, 0:128]
                    drhs = h["hid"][:, 0:TT]
                else:
                    dsl = h["dw"][:, ht * ICP + it * 128:
                                  ht * ICP + (it + 1) * 128]
                    drhs = h["hglu"][:, it * TT:(it + 1) * TT]
                nc.tensor.matmul(dsl if False else py[:], dsl, drhs,
                                 start=(it == 0), stop=(it == MT - 1))
            if h["nch"] % 2 == 0:
                nc.vector.tensor_copy(yo[:, bi * TT:(bi + 1) * TT], py[:])
            else:
                nc.scalar.copy(yo[:, bi * TT:(bi + 1) * TT], py[:])
            h["nch"] += 1
        if do_stores:
            dst = y_d[ht0 * 128:(ht0 + nb) * 128, :].rearrange(
                "(a p) t -> p a t", p=128)
            nc.scalar.dma_start(dst,
                                yo[:].rearrange("p (a t) -> p a t", a=nb))

    pe_shared = {}

    def emit_bodies(pools, n):
        prev = None
        for bi in range(n):
            if pe_mode:
                # loads once (first body); later bodies reuse the tiles but
                # still get a fresh hglu slot
                if not pe_shared:
                    globals()['_x'] = 0
                    pe_shared.update(emit_loads(pools, do_dma=True))
                h = dict(pe_shared)
                h["hglu"] = pools[2].tile([128, MT * TT], bf16,
                                          tag="hglu", name="hglu")
                h["nch"] = 0
            else:
                h = emit_loads(pools)
            if do_compute:
                for c in range(6):
                    emit_gu_chain(pools, h, c)
                    if mix and prev is not None:
                        emit_down_group(pools, prev, c)
                if not mix:
                    for g in range(6):
                        emit_down_group(pools, h, g)
                prev = h
        if mix and prev is not None and do_compute:
            for g in range(6):
                emit_down_group(pools, prev, g)

    from contextlib import ExitStack
    with tile.TileContext(nc) as tc:
        with ExitStack() as ctx:
            pools = (
                ctx.enter_context(tc.tile_pool(name="w", bufs=2)),
                ctx.enter_context(tc.tile_pool(name="glu", bufs=3)),
                ctx.enter_context(tc.tile_pool(name="hglu", bufs=2)),
                ctx.enter_context(tc.tile_pool(name="psum", bufs=2,
                                               space="PSUM")),
                ctx.enter_context(tc.tile_pool(name="psum_y", bufs=py_bufs,
                                               space="PSUM")),
                ctx.enter_context(tc.tile_pool(name="yout", bufs=3)),
            )
            if unroll_bodies is not None:
                emit_bodies(pools, unroll_bodies)
            elif loop_reps is None:
                emit_bodies(pools, 1)
            else:
                assert loop_reps % unroll == 0
                with tc.For_i(0, loop_reps // unroll, 1,
                              hint_engines=(mybir.EngineType.PE,)):
                    emit_bodies(pools, unroll)

    nc.compile()
    return nc


def prepare_in_maps(hidden_states, gate_w, gate_b, up_w, up_b, down_w,
                    active=None, tp=None):
    """Host-side shard + pad + pre-tile into the exact SBUF layouts.
    active/tp: token compaction — only hs[active] rows are shipped, padded
    to tp columns (defaults to all T tokens)."""
    hs = np.asarray(hidden_states, np.float32)
    if active is None:
        active = np.arange(T)
    if tp is None:
        tp = T
    hidT = np.zeros((HP, tp), np.float32)
    hidT[:H, :len(active)] = hs[active].T
    hid_tiled = np.ascontiguousarray(
        hidT.astype(BF16).reshape(KT, 128, tp).transpose(1, 0, 2)
    ).reshape(128, KT * tp)

    gw = np.asarray(gate_w, np.float32)
    uw = np.asarray(up_w, np.float32)
    dwf = np.asarray(down_w, np.float32)
    gbf = np.asarray(gate_b, np.float32).reshape(-1)
    ubf = np.asarray(up_b, np.float32).reshape(-1)

    def lhsT_tiles(Wp):  # [HP, 128] -> [128, KT*128]
        return np.ascontiguousarray(
            Wp.reshape(KT, 128, 128).transpose(1, 0, 2)).reshape(128, KT * 128)

    in_maps = []
    for c in range(NCORES):
        sl = slice(c * IC, (c + 1) * IC)
        Gp = np.zeros((HP, ICP), np.float32)
        Gp[:H, :IC] = gw[:, sl]
        Up = np.zeros((HP, ICP), np.float32)
        Up[:H, :IC] = uw[:, sl]
        Gp = Gp.astype(FP8)
        Up = Up.astype(FP8)
        blocks = []
        for m in range(MT):
            blocks.append(lhsT_tiles(Gp[:, m * 128:(m + 1) * 128]))
            blocks.append(lhsT_tiles(Up[:, m * 128:(m + 1) * 128]))
        gu = np.ascontiguousarray(np.concatenate(blocks, axis=1))

        Dp = np.zeros((ICP, HP), np.float32)
        Dp[:IC, :H] = dwf[sl, :]
        dw_tiled = np.ascontiguousarray(
            Dp.astype(FP8).reshape(MT, 128, KT, 128).transpose(1, 2, 0, 3)
        ).reshape(128, KT * MT * 128)

        gbp = np.zeros(ICP, np.float32)
        gbp[:IC] = gbf[sl]
        ubp = np.zeros(ICP, np.float32)
        ubp[:IC] = ubf[sl]

        in_maps.append({
            "hid": hid_tiled,
            "gu": gu,
            "dw": dw_tiled,
            "gb": np.ascontiguousarray(gbp.reshape(MT, 128).T),
            "ub": np.ascontiguousarray(ubp.reshape(MT, 128).T),
        })
    return in_maps


def kernel(hidden_states, routing_weights, final_hidden_states,
           gate_w, gate_b, up_w, up_b, down_w, down_b, expert_mask):
    from concourse.bass_utils import run_bass_kernel_spmd

    active, tok_w, tp = routing_compaction(expert_mask, routing_weights)
    out = np.array(np.asarray(final_hidden_states, np.float32), copy=True)
    if len(active) == 0:
        return out.astype(np.float32)

    if tp not in _cache:
        _cache[tp] = build_program(tp=tp)
    nc = _cache[tp]

    in_maps = prepare_in_maps(hidden_states, gate_w, gate_b, up_w, up_b,
                              down_w, active, tp)
    res = run_bass_kernel_spmd(nc, in_maps, list(range(NCORES)))

    ysum = np.zeros((HP, tp), np.float64)
    for c in range(NCORES):
        ysum += res.results[c]["y"].astype(np.float64)
    y = ysum[:H, :len(active)].T.astype(np.float32)     # [n_active, H]

    out[active] += ((y + np.asarray(down_b, np.float32).reshape(1, -1))
                    * tok_w[active, None])
    return out.astype(np.float32)

